# revision 1
# baseline (speedup 1.0000x reference)
"""Two-layer SAGEConv (mean aggregation) GNN on 8 trn2 NeuronCores.

Strategy (dst-sharded graph parallel, "paired gather"):
  - dst nodes are packed into 320 bins of 128 slots (40 bins per core) by a
    degree-balanced snake-LPT, so every bin holds <= 2048 edges. Each bin is
    one psum "range".
  - Each core gets its OWN permuted pair-table [32768, 256] bf16: row k holds
    features of TWO source nodes (A-half cols 0:128, B-half cols 128:256).
    One 512-byte gather descriptor therefore serves up to two edges, and
    512B descriptors avoid the <512B 2x DMA latency penalty. Packing is
    two-phase: (1) sources are globally paired on their first-two-bins key,
    sharing slots wherever both land in a bin; (2) leftover lone edges of a
    bin are paired with each other via freshly created rows — features are
    simply duplicated in the DRAM table, spending the spare int16 row budget
    to reach ~1.97 edges per descriptor (within 2.3% of the 2-edge ceiling).
  - Per-bin slot counts are uneven, so bins are relabeled per core in
    descending slot count and the program is compiled for the shared
    per-range block-count profile (max across cores).
  - Routing matrices are generated ON-CHIP from slot-target bytes via
    broadcast is_equal on DVE in a slot-major layout (keeps last-dim
    stride-1 so the DVE 2x perf mode applies); generation is issued two
    gather-groups ahead and each group's lin/act phase is deferred behind
    the next group's agg matmuls, so neither the in-order DVE nor PE queue
    head-of-line blocks during the post-gather drain.
  - Aggregation per range: psum[f, slot] += sum_j msg[:, j, 0:128].T @ RA_j
    + msg[:, j, 128:256].T @ RB_j; then mean scale (1/deg along free dim),
    outT = act(W_l.T @ meanT + W_r.T @ xT + b), streamed out per group as
    feature-major [128, 5120] bf16 (the host transposes / re-permutes the h
    table between the two layer launches).
"""
import numpy as np
import ml_dtypes
from contextlib import ExitStack

import concourse.bass as bass
import concourse.mybir as mybir
import concourse.tile as tile
from concourse import bacc
from concourse.library_config import mlp
from concourse import bass_utils

BF16 = mybir.dt.bfloat16
F32 = mybir.dt.float32
I16 = mybir.dt.int16
NP_BF16 = ml_dtypes.bfloat16

N = 40000
D = 128
CORES = 8
RANGES = 40            # bins (dst ranges of 128 slots) per core
NPAD = RANGES * 128    # 5120 dst positions per core
NBINS = CORES * RANGES
PAIR_ROWS = 32768      # pair-table rows (int16-indexable)
GMAX = 24              # max message blocks per gather group (SBUF budget)

_prog_cache = {}


def _make_groups(KP):
    """Split program ranges into gather groups of <= GMAX blocks.
    Returns list of (range_lo, range_hi, block_offset, nblk). The final
    two groups are single ranges to shorten the post-gather drain."""
    groups = []
    lo = 0
    off = 0
    cur = 0
    for r in range(RANGES - 2):
        if cur + KP[r] > GMAX and cur > 0:
            groups.append((lo, r, off, cur))
            off += cur
            lo = r
            cur = 0
        cur += KP[r]
    groups.append((lo, RANGES - 2, off, cur))
    off += cur
    groups.append((RANGES - 2, RANGES - 1, off, KP[RANGES - 2]))
    off += KP[RANGES - 2]
    groups.append((RANGES - 1, RANGES, off, KP[RANGES - 1]))
    return groups


def build_program(layer, KP):
    """One SPMD program for one SAGEConv layer. KP[r] = pair-blocks of range r."""
    KP = list(KP)
    TOTBLK = sum(KP)
    IDX_COLS = TOTBLK * 8          # idx wrap cols (num_idxs/16 per call, concat)
    groups = _make_groups(KP)
    starts = np.concatenate([[0], np.cumsum(KP)]).astype(int)

    nc = bacc.Bacc("TRN2", target_bir_lowering=False, debug=False)
    table = nc.dram_tensor("table", [PAIR_ROWS, 2 * D], BF16, kind="ExternalInput")
    idx_d = nc.dram_tensor("idxs", [128, IDX_COLS], I16, kind="ExternalInput")
    tgtA_d = nc.dram_tensor("tgtA", [128, TOTBLK], BF16, kind="ExternalInput")
    tgtB_d = nc.dram_tensor("tgtB", [128, TOTBLK], BF16, kind="ExternalInput")
    iota_d = nc.dram_tensor("iota", [128, 128], BF16, kind="ExternalInput")
    xT_d = nc.dram_tensor("xT", [128, NPAD], BF16, kind="ExternalInput")
    recip_d = nc.dram_tensor("recipb", [1, NPAD], BF16, kind="ExternalInput")
    Wl_d = nc.dram_tensor("Wl", [128, 128], BF16, kind="ExternalInput")
    Wr_d = nc.dram_tensor("Wr", [128, 128], BF16, kind="ExternalInput")
    b_d = nc.dram_tensor("bvec", [128, 1], F32, kind="ExternalInput")
    # feature-major [f, pos]: host transposes (it re-permutes tables anyway)
    tout = nc.dram_tensor("tout", [128, NPAD], BF16, kind="ExternalOutput")

    with tile.TileContext(nc) as tc, ExitStack() as ctx:
        const = ctx.enter_context(tc.tile_pool(name="const", bufs=1))
        pmsg = ctx.enter_context(tc.tile_pool(name="msg", bufs=9))
        pohA = ctx.enter_context(tc.tile_pool(name="ohA", bufs=2))
        pohB = ctx.enter_context(tc.tile_pool(name="ohB", bufs=2))
        psagg = ctx.enter_context(tc.tile_pool(name="psagg", bufs=5, space="PSUM"))
        pslin = ctx.enter_context(tc.tile_pool(name="pslin", bufs=2, space="PSUM"))
        pmean = ctx.enter_context(tc.tile_pool(name="mean", bufs=10))
        psbc = ctx.enter_context(tc.tile_pool(name="psbc", bufs=1, space="PSUM"))

        nc.gpsimd.load_library(mlp)

        idxs = const.tile([128, IDX_COLS], I16)
        c0 = groups[0][3] * 8
        nc.sync.dma_start(idxs[:, :c0], idx_d[:, :c0])
        nc.sync.dma_start(idxs[:, c0:], idx_d[:, c0:])
        tgtA = const.tile([128, TOTBLK], BF16)
        nc.sync.dma_start(tgtA[:], tgtA_d[:])
        tgtB = const.tile([128, TOTBLK], BF16)
        nc.sync.dma_start(tgtB[:], tgtB_d[:])
        iota = const.tile([128, 128], BF16)
        nc.sync.dma_start(iota[:], iota_d[:])
        # iotaE[p, f, b] = f — stride-1 last dims for the DVE 2x perf mode
        iotaE = const.tile([128, 128, GMAX], BF16)
        nc.vector.tensor_copy(iotaE[:],
                              iota[:, :, None].to_broadcast([128, 128, GMAX]))
        xT = const.tile([128, NPAD], BF16)
        nc.sync.dma_start(xT[:], xT_d[:])
        # recip is a partition-broadcast of a 10KB row: fetch one row and
        # replicate it across partitions with K=1 matmuls against a ones
        # stationary (PE + Act are idle; DMA is the bottleneck).
        recipRow = const.tile([1, NPAD], BF16)
        nc.sync.dma_start(recipRow[:], recip_d[:])
        ones1 = const.tile([1, 128], BF16)
        nc.vector.memset(ones1[:], 1.0)
        zv = const.tile([128, 1], F32)
        nc.vector.memset(zv[:], 0.0)
        recip = const.tile([128, NPAD], BF16)
        for t in range(NPAD // 512):
            psb = psbc.tile([128, 512], F32)
            nc.tensor.matmul(psb[:], ones1[:],
                             recipRow[:, t * 512:(t + 1) * 512],
                             start=True, stop=True)
            nc.scalar.activation(recip[:, t * 512:(t + 1) * 512], psb[:],
                                 mybir.ActivationFunctionType.Identity,
                                 bias=zv[:])
        Wl = const.tile([128, 128], BF16)
        nc.sync.dma_start(Wl[:], Wl_d[:])
        Wr = const.tile([128, 128], BF16)
        nc.sync.dma_start(Wr[:], Wr_d[:])
        bv = const.tile([128, 1], F32)
        nc.sync.dma_start(bv[:], b_d[:])
        ostage = const.tile([128, NPAD], BF16)

        def gen_oh(pool_or_const, tgt, boff, nblk):
            oh = pool_or_const.tile([128, 128, nblk], BF16)
            nc.vector.tensor_tensor(
                out=oh[:],
                in0=tgt[:, None, boff:boff + nblk]
                .to_broadcast([128, 128, nblk]),
                in1=iotaE[:, :, :nblk],
                op=mybir.AluOpType.is_equal)
            return oh

        def emit_lin(rlo, rhi, means):
            for i, r in enumerate(range(rlo, rhi)):
                ps2 = pslin.tile([128, 128], F32)
                nc.tensor.matmul(ps2[:], Wl[:], means[i][:],
                                 start=True, stop=False)
                nc.tensor.matmul(ps2[:], Wr[:], xT[:, r * 128:(r + 1) * 128],
                                 start=False, stop=True)
                nc.scalar.activation(
                    ostage[:, r * 128:(r + 1) * 128], ps2[:],
                    mybir.ActivationFunctionType.Relu if layer == 1
                    else mybir.ActivationFunctionType.Identity,
                    bias=bv[:])
            # stream this group's outputs so the final write doesn't
            # serialize after the last gather
            nc.sync.dma_start(tout[:, rlo * 128:rhi * 128],
                              ostage[:, rlo * 128:rhi * 128])

        lin_work = None
        # DVE executes in issue order: generate routing matrices two groups
        # AHEAD of the compute that consumes the previous group's psums, so
        # gen is never head-of-line blocked behind the mean ops and the last
        # group's matmuls can start the moment its gather lands.
        from collections import deque
        pending = deque()
        for gi in range(min(2, len(groups))):
            g = groups[gi]
            pending.append((gen_oh(pohA, tgtA, g[2], g[3]),
                            gen_oh(pohB, tgtB, g[2], g[3])))
        for gi, (rlo, rhi, boff, nblk) in enumerate(groups):
            GN = nblk * 128
            msg = pmsg.tile([128, nblk, 2 * D], BF16)
            nc.gpsimd.dma_gather(msg[:], table[:, :],
                                 idxs[:, boff * 8:(boff + nblk) * 8],
                                 GN, GN, 2 * D, single_packet=False)
            ohA, ohB = pending.popleft()
            if gi + 2 < len(groups):
                nb = groups[gi + 2]
                pending.append((gen_oh(pohA, tgtA, nb[2], nb[3]),
                                gen_oh(pohB, tgtB, nb[2], nb[3])))

            # phase-batched + software-pipelined: issue this group's agg
            # matmuls and means now, but defer its lin/act/write phase until
            # after the NEXT group's agg matmuls are in the (in-order) PE
            # queue — so agg work never stalls behind lins waiting on DVE.
            pss = []
            for r in range(rlo, rhi):
                k = KP[r]
                ps = psagg.tile([128, 128], F32)
                for j in range(k):
                    bb = starts[r] - boff + j
                    nc.tensor.matmul(ps[:], msg[:, bb, 0:D],
                                     ohA[:, :, bb],
                                     start=(j == 0), stop=False)
                    nc.tensor.matmul(ps[:], msg[:, bb, D:2 * D],
                                     ohB[:, :, bb],
                                     start=False, stop=(j == k - 1))
                pss.append(ps)
            means = []
            for i, r in enumerate(range(rlo, rhi)):
                mean = pmean.tile([128, 128], BF16)
                nc.vector.tensor_mul(mean[:], pss[i][:],
                                     recip[:, r * 128:(r + 1) * 128])
                means.append(mean)
            if lin_work is not None:
                emit_lin(*lin_work)
            lin_work = (rlo, rhi, means)
        emit_lin(*lin_work)
    nc.compile()
    return nc


def _wrap_idxs(streams):
    """list of per-call idx streams (len % 2048 == 0) -> [128, sum/16] int16
    sbuf wrap layout (16-partition wrap per call, replicated to 128)."""
    cols = []
    for s in streams:
        a = s.reshape(-1, 16).T  # [16, GN/16]
        cols.append(a)
    a = np.concatenate(cols, axis=1)
    return np.tile(a, (8, 1)).astype(np.int16)


def _bin_nodes(deg):
    """Degree-balanced assignment of nodes to 320 bins of 128 slots."""
    order = np.argsort(-deg, kind="stable")
    loads = np.zeros(NBINS, np.int64)
    bin_of_node = np.empty(N, np.int64)
    slot_of_node = np.empty(N, np.int64)
    nrounds = (N + NBINS - 1) // NBINS
    for rnd in range(nrounds):
        chunk = order[rnd * NBINS:(rnd + 1) * NBINS]
        border = np.argsort(loads, kind="stable")[:len(chunk)]
        bin_of_node[chunk] = border
        slot_of_node[chunk] = rnd
        loads[border] += deg[chunk]
    return bin_of_node, slot_of_node, loads


def _ranks(rows):
    """Per-element rank within equal-value group of sorted-by-value `rows`,
    plus unique values and counts. rows need not be sorted."""
    o = np.argsort(rows, kind="stable")
    sr = rows[o]
    if len(sr) == 0:
        return np.empty(0, np.int64), np.empty(0, np.int64), np.empty(0, np.int64)
    newgrp = np.r_[True, sr[1:] != sr[:-1]]
    starts = np.flatnonzero(newgrp)
    grp = np.cumsum(newgrp) - 1
    pos = np.arange(len(sr)) - starts[grp]
    rank = np.empty(len(rows), np.int64)
    rank[o] = pos
    ur = sr[starts]
    cnt = np.diff(np.r_[starts, len(sr)])
    return rank, ur, cnt


def _greedy_pair(keys):
    """Pair elements (indices) having equal keys: returns (a_idx, b_idx,
    leftover_idx). Elements are paired consecutively within equal-key runs."""
    o = np.argsort(keys, kind="stable")
    ks = keys[o]
    if len(ks) == 0:
        z = np.empty(0, np.int64)
        return z, z, z
    newg = np.r_[True, ks[1:] != ks[:-1]]
    starts = np.flatnonzero(newg)
    gid = np.cumsum(newg) - 1
    pos = np.arange(len(ks)) - starts[gid]
    sizes = np.diff(np.r_[starts, len(ks)])
    odd_last = (pos == sizes[gid] - 1) & (sizes[gid] % 2 == 1)
    paired = ~odd_last
    po = o[paired]
    return po[0::2], po[1::2], o[odd_last]


def _pair_sources(src_c, rloc_c):
    """Global pairing of this core's sources by their first-two-bins key:
    a pair sharing two bins saves a gather slot in both. Returns
    (assignment [N] in {-1,0,1}, rowof [N], rows_used)."""
    key = src_c * 64 + rloc_c
    ub = np.unique(key)
    usrc, ubin = ub >> 6, ub & 63
    first = np.r_[True, usrc[1:] != usrc[:-1]]
    idx_first = np.flatnonzero(first)
    srcs_u = usrc[idx_first]
    b1 = ubin[idx_first]
    nxt_is_same = np.r_[idx_first[1:] - idx_first[:-1] > 1,
                        len(ub) - idx_first[-1] > 1]
    b2 = np.where(nxt_is_same, ubin[np.minimum(idx_first + 1, len(ub) - 1)], 64)
    # round 1: match on (first bin, second bin); round 2: leftovers on b1
    a1, bb1, left = _greedy_pair(b1 * 65 + b2)
    a2, bb2, left2 = _greedy_pair(b1[left])
    a_i = np.concatenate([a1, left[a2]])
    b_i = np.concatenate([bb1, left[bb2]])
    single = left[left2]

    assignment = np.full(N, -1, np.int8)
    rowof = np.zeros(N, np.int32)
    npairs = len(a_i)
    assignment[srcs_u[a_i]] = 0
    rowof[srcs_u[a_i]] = np.arange(npairs)
    assignment[srcs_u[b_i]] = 1
    rowof[srcs_u[b_i]] = np.arange(len(b_i))
    assignment[srcs_u[single]] = 0
    rowof[srcs_u[single]] = npairs + np.arange(len(single))
    return assignment, rowof, npairs + len(single)


def _core_streams(src_c, rloc_c, slot_c):
    """Per-core pair assignment + per-bin slot streams.

    Phase 1: sources get a primary row/half from _pair_sources; within a bin
    the i-th A-edge and i-th B-edge of the same row share a slot.
    Phase 2: leftover lone edges of a bin are paired with each other via
    NEWLY CREATED rows (the sources' features are simply duplicated in the
    DRAM table — row budget is the int16 range), halving their slot count.

    Returns (streams: list of (idx_r, tgtA_r, tgtB_r) per physical bin,
    nslots [RANGES], rowA [PAIR_ROWS], rowB [PAIR_ROWS] source ids)."""
    assignment, rowof, rows_used = _pair_sources(src_c, rloc_c)
    rowA = np.full(PAIR_ROWS, -1, np.int64)
    rowB = np.full(PAIR_ROWS, -1, np.int64)
    nodesA = np.where(assignment == 0)[0]
    nodesB = np.where(assignment == 1)[0]
    rowA[rowof[nodesA]] = nodesA
    rowB[rowof[nodesB]] = nodesB
    nxt = rows_used
    created = {}
    streams = []
    nslots = np.zeros(RANGES, np.int64)

    order = np.argsort(rloc_c, kind="stable")
    src_s = src_c[order]
    slot_s = slot_c[order]
    bounds = np.searchsorted(rloc_c[order], np.arange(RANGES + 1))
    for r in range(RANGES):
        lo, hi = bounds[r], bounds[r + 1]
        s = src_s[lo:hi]
        sl = slot_s[lo:hi]
        half_e = assignment[s]
        rows_e = rowof[s].astype(np.int64)
        # rank of each edge within its (row, half) group
        kh = rows_e * 2 + half_e
        rank_e, ur_kh, cnt_kh = _ranks(kh)
        # per-row counts on each half
        ur = np.unique(rows_e)
        cA = np.zeros(len(ur), np.int64)
        cB = np.zeros(len(ur), np.int64)
        ia = np.searchsorted(ur, ur_kh[ur_kh % 2 == 0] // 2)
        cA[ia] = cnt_kh[ur_kh % 2 == 0]
        ib = np.searchsorted(ur, ur_kh[ur_kh % 2 == 1] // 2)
        cB[ib] = cnt_kh[ur_kh % 2 == 1]
        full = np.minimum(cA, cB)
        base = np.r_[0, np.cumsum(full)[:-1]]
        nfull = int(full.sum())
        ri = np.searchsorted(ur, rows_e)
        is_full = rank_e < full[ri]
        # emit full slots
        nlone = int((~is_full).sum())
        npair2 = (nlone + 1) // 2
        ns = nfull + npair2
        idx_r = np.zeros(ns, np.int16)
        tgtA_r = np.full(ns, 255, np.int16)
        tgtB_r = np.full(ns, 255, np.int16)
        slot_of_e = np.empty(len(s), np.int64)
        half_of_e = np.empty(len(s), np.int64)
        slot_of_e[is_full] = base[ri[is_full]] + rank_e[is_full]
        half_of_e[is_full] = half_e[is_full]
        idx_r[base[ri[is_full]] + rank_e[is_full]] = rows_e[is_full]
        # phase 2: pair lone edges via created rows
        lone_i = np.where(~is_full)[0]
        for j in range(0, len(lone_i) - 1, 2):
            e1, e2 = lone_i[j], lone_i[j + 1]
            u, v = int(s[e1]), int(s[e2])
            k = created.get((u, v))
            if k is not None:
                h1, h2 = 0, 1
            else:
                k = created.get((v, u))
                if k is not None:
                    h1, h2 = 1, 0
                elif nxt < PAIR_ROWS:
                    k = nxt
                    created[(u, v)] = k
                    rowA[k] = u
                    rowB[k] = v
                    nxt += 1
                    h1, h2 = 0, 1
                else:
                    # row budget exhausted: fall back to primary rows
                    sp = nfull + j // 2
                    slot_of_e[e1] = sp
                    half_of_e[e1] = half_e[e1]
                    idx_r[sp] = rows_e[e1]
                    # e2 shares nothing; give it the next phase-2 slot? no
                    # spare — stack on same slot only if rows match; else
                    # this should not happen for this problem size.
                    raise OverflowError("pair row budget exhausted")
            sp = nfull + j // 2
            slot_of_e[e1] = sp
            half_of_e[e1] = h1
            slot_of_e[e2] = sp
            half_of_e[e2] = h2
            idx_r[sp] = k
        if len(lone_i) % 2 == 1:
            e1 = lone_i[-1]
            sp = ns - 1
            slot_of_e[e1] = sp
            half_of_e[e1] = half_e[e1]
            idx_r[sp] = rows_e[e1]
        hA = half_of_e == 0
        tgtA_r[slot_of_e[hA]] = sl[hA]
        tgtB_r[slot_of_e[~hA]] = sl[~hA]
        streams.append((idx_r, tgtA_r, tgtB_r))
        nslots[r] = ns
    return streams, nslots, rowA, rowB


def preprocess(x, edge_index):
    src = np.asarray(edge_index[0], dtype=np.int64)
    dst = np.asarray(edge_index[1], dtype=np.int64)
    deg = np.bincount(dst, minlength=N)
    recip = (1.0 / np.maximum(deg, 1)).astype(np.float32)

    bin_of_node, slot_of_node, loads = _bin_nodes(deg)
    ecore = bin_of_node[dst] // RANGES
    erloc = bin_of_node[dst] % RANGES
    eslot = slot_of_node[dst]

    xv = np.asarray(x, dtype=np.float32)
    per_core = []
    nslots_all = np.zeros((CORES, RANGES), np.int64)
    for c in range(CORES):
        m = ecore == c
        streams, nslots, rowA, rowB = _core_streams(
            src[m], erloc[m], eslot[m])
        per_core.append((streams, nslots, rowA, rowB))
        nslots_all[c] = nslots

    # per-core bin relabel (desc slot count) + shared block-count profile
    perms = [np.argsort(-nslots_all[c], kind="stable") for c in range(CORES)]
    sorted_ns = np.stack([nslots_all[c][perms[c]] for c in range(CORES)])
    profile = sorted_ns.max(axis=0)
    KP = np.maximum(np.ceil(profile / 128).astype(int), 1)
    if profile.max() > 2048:
        raise OverflowError(f"range overflow {profile.max()}")
    TOTBLK = int(KP.sum())
    groups = _make_groups(list(KP))

    cores = []
    for c in range(CORES):
        streams, nslots, rsrcA, rsrcB = per_core[c]
        perm = perms[c]
        idx_full = np.zeros((TOTBLK * 128,), np.int16)
        tgtA_full = np.full((TOTBLK * 128,), 255, np.int16)
        tgtB_full = np.full((TOTBLK * 128,), 255, np.int16)
        off = 0
        for r in range(RANGES):
            idx_r, tgtA_r, tgtB_r = streams[perm[r]]
            ns = len(idx_r)
            idx_full[off:off + ns] = idx_r
            tgtA_full[off:off + ns] = tgtA_r
            tgtB_full[off:off + ns] = tgtB_r
            off += KP[r] * 128
        call_streams = [idx_full[boff * 128:(boff + nblk) * 128]
                        for (_, _, boff, nblk) in groups]
        wrap = _wrap_idxs(call_streams)
        tgtA = np.ascontiguousarray(
            tgtA_full.reshape(TOTBLK, 128).T.astype(np.float32)).astype(NP_BF16)
        tgtB = np.ascontiguousarray(
            tgtB_full.reshape(TOTBLK, 128).T.astype(np.float32)).astype(NP_BF16)

        # own nodes in relabeled pos order
        own = np.full(NPAD, -1, np.int64)
        mc = bin_of_node // RANGES == c
        nodes_c = np.where(mc)[0]
        rinv = np.empty(RANGES, np.int64)
        rinv[perm] = np.arange(RANGES)
        own[rinv[bin_of_node[nodes_c] % RANGES] * 128
            + slot_of_node[nodes_c]] = nodes_c
        cores.append(dict(wrap=wrap, tgtA=tgtA, tgtB=tgtB,
                          rsrcA=rsrcA, rsrcB=rsrcB, own=own))

    def table_from(feats_by_node):
        out = []
        for c in range(CORES):
            t = np.zeros((PAIR_ROWS, 2 * D), NP_BF16)
            for half, key in ((0, "rsrcA"), (1, "rsrcB")):
                rs = cores[c][key]
                used = rs >= 0
                t[used, half * D:(half + 1) * D] = \
                    feats_by_node[rs[used]].astype(NP_BF16)
            out.append(t)
        return out

    xT = []
    recipb = []
    for c in range(CORES):
        own = cores[c]["own"]
        used = own >= 0
        t = np.zeros((NPAD, D), np.float32)
        t[used] = xv[own[used]]
        xT.append(np.ascontiguousarray(t.T).astype(NP_BF16))
        rb = np.zeros((NPAD,), np.float32)
        rb[used] = recip[own[used]]
        recipb.append(rb.astype(NP_BF16).reshape(1, NPAD))

    return cores, table_from, xT, recipb, tuple(KP.tolist()), xv


def kernel(x, edge_index, W1_l, b1, W1_r, W2_l, b2, W2_r, _timing=None):
    cores, table_from, xT, recipb, KP, xv = preprocess(x, edge_index)

    if KP not in _prog_cache:
        _prog_cache[KP] = (build_program(1, KP), build_program(2, KP))
    nc1, nc2 = _prog_cache[KP]

    def wmat(w):
        return np.asarray(w, dtype=np.float32).astype(NP_BF16)

    def bcol(b):
        return np.asarray(b, dtype=np.float32).reshape(128, 1)

    iota = np.ascontiguousarray(
        np.broadcast_to(np.arange(128, dtype=np.float32), (128, 128))
    ).astype(NP_BF16)
    tables1 = table_from(xv)
    maps1 = []
    for c in range(CORES):
        cc = cores[c]
        maps1.append(dict(table=tables1[c], idxs=cc["wrap"],
                          tgtA=cc["tgtA"], tgtB=cc["tgtB"], iota=iota,
                          xT=xT[c], recipb=recipb[c], Wl=wmat(W1_l),
                          Wr=wmat(W1_r), bvec=bcol(b1)))
    r1 = bass_utils.run_bass_kernel_spmd(nc1, maps1, core_ids=list(range(CORES)))

    # h by global node id (houts are feature-major in relabeled pos order)
    h_node = np.zeros((N, D), np.float32)
    for c in range(CORES):
        own = cores[c]["own"]
        used = own >= 0
        h_node[own[used]] = r1.results[c]["tout"].T[used]
    tables2 = table_from(h_node)

    maps2 = []
    for c in range(CORES):
        cc = cores[c]
        hT_own = np.asarray(r1.results[c]["tout"], dtype=np.float32).astype(NP_BF16)
        maps2.append(dict(table=tables2[c], idxs=cc["wrap"],
                          tgtA=cc["tgtA"], tgtB=cc["tgtB"], iota=iota,
                          xT=hT_own, recipb=recipb[c], Wl=wmat(W2_l),
                          Wr=wmat(W2_r), bvec=bcol(b2)))
    r2 = bass_utils.run_bass_kernel_spmd(nc2, maps2, core_ids=list(range(CORES)))
    if _timing is not None:
        _timing["nc1"] = nc1
        _timing["nc2"] = nc2

    out = np.empty((N, D), np.float32)
    for c in range(CORES):
        own = cores[c]["own"]
        used = own >= 0
        out[own[used]] = r2.results[c]["tout"].T[used]
    return out



# revision 3
# speedup vs baseline: 1.5116x; 1.5116x over previous
"""Two-layer SAGEConv (mean aggregation) GNN on 8 trn2 NeuronCores.

Strategy (dst-sharded graph parallel, "fp8 quad bundles"):
  - dst nodes are assigned to cores by LPT on bundle count, then bin-packed
    per core into ranges of <=128 nodes and <=512 bundles (4 psum blocks).
  - A bundle is one 512-byte DRAM table row holding FOUR source feature
    vectors in fp8 e4m3, all belonging to edges of the SAME dst node, with
    recip(deg) pre-folded into the stored values. One gather descriptor
    therefore serves up to 4 edges at full-rate DMA (512B avoids the <512B
    2x latency penalty), and the psum accumulates the mean directly.
  - Because all 4 lanes of a bundle share one dst, each 128-slot block needs
    only ONE one-hot routing matrix (generated on-chip by DVE is_equal in
    fp8), shared by the 4 lane matmuls.
  - Aggregation runs as fp8xfp8 DoubleRow matmuls (two 128-slot blocks per
    matmul, 0.5 cycles/row): psum[f, dst] += sum msg_lane.T @ oh.
  - Act engine copies psum->bf16 mean; lin phase = Wl.T@mean + Wr.T@xT with
    bias+activation fused on Act; outputs stream out per group.
  - Routing matrices are generated two gather-groups ahead and each group's
    lin phase is deferred behind the next group's agg matmuls so neither the
    in-order DVE nor PE queue head-of-line blocks.
  - Only the gathered tables are fp8; xT/W/mean/outputs stay bf16
    (measured rel err ~1.3e-2 vs the 2e-2 gate).
"""
import numpy as np
import ml_dtypes
from contextlib import ExitStack
from collections import deque

import concourse.bass as bass
import concourse.mybir as mybir
import concourse.tile as tile
from concourse import bacc
from concourse.library_config import mlp
from concourse import bass_utils

BF16 = mybir.dt.bfloat16
F32 = mybir.dt.float32
F8 = mybir.dt.float8e4
I16 = mybir.dt.int16
NP_BF16 = ml_dtypes.bfloat16
NP_F8 = ml_dtypes.float8_e4m3

N = 40000
D = 128
CORES = 8
LANES = 4
BPR = 4                 # blocks per range
SLOTS_PER_RANGE = BPR * 128
CAP_NODES = 128         # dst nodes per range
ROWS = 23040            # gather-table row budget (int16-indexable)

_prog_cache = {}


def _make_groups(R):
    """Split R ranges into gather calls: small first call to start the DMA
    pipeline early, small final calls to shorten the drain."""
    sizes = []
    rem = R
    for s in (1, 4):
        if rem > s:
            sizes.append(s)
            rem -= s
    while rem > 3:
        sizes.append(min(5, rem - 3))
        rem -= sizes[-1]
    if rem == 3:
        sizes += [2, 1]
    elif rem > 0:
        sizes.append(rem)
    groups = []
    lo = 0
    for s in sizes:
        groups.append((lo, lo + s, lo * BPR, s * BPR))
        lo += s
    return groups


def build_program(layer, RANGES):
    """One SPMD program for one SAGEConv layer. Uniform BPR blocks/range."""
    TOTBLK = RANGES * BPR
    NPAD = RANGES * 128
    IDX_COLS = TOTBLK * 8
    groups = _make_groups(RANGES)

    nc = bacc.Bacc("TRN2", target_bir_lowering=False, debug=False)
    table = nc.dram_tensor("table", [ROWS, LANES * D], F8, kind="ExternalInput")
    idx_d = nc.dram_tensor("idxs", [128, IDX_COLS], I16, kind="ExternalInput")
    tgt_d = nc.dram_tensor("tgt", [128, TOTBLK], BF16, kind="ExternalInput")
    iota_d = nc.dram_tensor("iota", [128, 128], BF16, kind="ExternalInput")
    xT_d = nc.dram_tensor("xT", [128, NPAD], BF16, kind="ExternalInput")
    Wl_d = nc.dram_tensor("Wl", [128, 128], BF16, kind="ExternalInput")
    Wr_d = nc.dram_tensor("Wr", [128, 128], BF16, kind="ExternalInput")
    b_d = nc.dram_tensor("bvec", [128, 1], F32, kind="ExternalInput")
    # feature-major [f, pos]: host transposes (it re-permutes tables anyway)
    tout = nc.dram_tensor("tout", [128, NPAD], BF16, kind="ExternalOutput")

    with tile.TileContext(nc) as tc, ExitStack() as ctx:
        const = ctx.enter_context(tc.tile_pool(name="const", bufs=1))
        pmsg = ctx.enter_context(tc.tile_pool(name="msg", bufs=5))
        poh = ctx.enter_context(tc.tile_pool(name="oh", bufs=4))
        psagg = ctx.enter_context(tc.tile_pool(name="psagg", bufs=6, space="PSUM"))
        pslin = ctx.enter_context(tc.tile_pool(name="pslin", bufs=2, space="PSUM"))
        pmean = ctx.enter_context(tc.tile_pool(name="mean", bufs=12))

        nc.gpsimd.load_library(mlp)

        # small loads first so DMA goes busy immediately and the first
        # gather's inputs (idx group 0, tgt, iota) land early
        idxs = const.tile([128, IDX_COLS], I16)
        c0 = groups[0][3] * 8
        nc.sync.dma_start(idxs[:, :c0], idx_d[:, :c0])
        tgt = const.tile([128, TOTBLK], BF16)
        nc.sync.dma_start(tgt[:], tgt_d[:])
        iota = const.tile([128, 128], BF16)
        nc.sync.dma_start(iota[:], iota_d[:])
        nc.sync.dma_start(idxs[:, c0:], idx_d[:, c0:])
        Wl = const.tile([128, 128], BF16)
        nc.sync.dma_start(Wl[:], Wl_d[:])
        Wr = const.tile([128, 128], BF16)
        nc.sync.dma_start(Wr[:], Wr_d[:])
        bv = const.tile([128, 1], F32)
        nc.sync.dma_start(bv[:], b_d[:])
        xT = const.tile([128, NPAD], BF16)
        nc.sync.dma_start(xT[:], xT_d[:])
        zv = const.tile([128, 1], F32)
        nc.vector.memset(zv[:], 0.0)
        ostage = const.tile([128, NPAD], BF16)

        def gen_oh(boff, nblk):
            # oh[p, b, dst] = (tgt[p, boff+b] == dst), shared by all 4 lanes
            oh = poh.tile([128, nblk, 128], F8)
            nc.vector.tensor_tensor(
                out=oh[:],
                in0=tgt[:, boff:boff + nblk, None]
                .to_broadcast([128, nblk, 128]),
                in1=iota[:, None, :].to_broadcast([128, nblk, 128]),
                op=mybir.AluOpType.is_equal)
            return oh

        def emit_lin(rlo, rhi, means):
            for i, r in enumerate(range(rlo, rhi)):
                ps2 = pslin.tile([128, 128], F32)
                nc.tensor.matmul(ps2[:], Wl[:], means[i][:],
                                 start=True, stop=False)
                nc.tensor.matmul(ps2[:], Wr[:], xT[:, r * 128:(r + 1) * 128],
                                 start=False, stop=True)
                nc.scalar.activation(
                    ostage[:, r * 128:(r + 1) * 128], ps2[:],
                    mybir.ActivationFunctionType.Relu if layer == 1
                    else mybir.ActivationFunctionType.Identity,
                    bias=bv[:])
            nc.sync.dma_start(tout[:, rlo * 128:rhi * 128],
                              ostage[:, rlo * 128:rhi * 128])

        lin_work = None
        pending = deque()
        for gi in range(min(2, len(groups))):
            g = groups[gi]
            pending.append(gen_oh(g[2], g[3]))
        for gi, (rlo, rhi, boff, nblk) in enumerate(groups):
            GN = nblk * 128
            msg = pmsg.tile([128, nblk, LANES * D], F8)
            nc.gpsimd.dma_gather(msg[:], table[:, :],
                                 idxs[:, boff * 8:(boff + nblk) * 8],
                                 GN, GN, LANES * D, single_packet=False)
            oh = pending.popleft()
            if gi + 2 < len(groups):
                nb = groups[gi + 2]
                pending.append(gen_oh(nb[2], nb[3]))

            means = []
            for r in range(rlo, rhi):
                bb = (r - rlo) * BPR
                ps = psagg.tile([128, 128], F32)
                n = 0
                last = (BPR // 2) * LANES - 1
                for j in range(BPR // 2):
                    for lane in range(LANES):
                        nc.tensor.matmul(
                            ps[:],
                            msg[:, bb + 2 * j:bb + 2 * j + 2,
                                lane * D:(lane + 1) * D],
                            oh[:, bb + 2 * j:bb + 2 * j + 2, :],
                            start=(n == 0), stop=(n == last),
                            perf_mode=mybir.MatmulPerfMode.DoubleRow)
                        n += 1
                # psum already holds the mean (recip folded into the table);
                # Act engine copies it to bf16 for the lin moving operand
                mean = pmean.tile([128, 128], BF16)
                nc.scalar.activation(mean[:], ps[:],
                                     mybir.ActivationFunctionType.Identity,
                                     bias=zv[:])
                means.append(mean)
            if lin_work is not None:
                emit_lin(*lin_work)
            lin_work = (rlo, rhi, means)
        emit_lin(*lin_work)
    nc.compile()
    return nc


def _wrap_idxs(streams):
    """list of per-call idx streams (len % 16 == 0) -> [128, sum/16] int16
    sbuf wrap layout (16-partition wrap per call, replicated to 128)."""
    cols = []
    for s in streams:
        cols.append(s.reshape(-1, 16).T)
    a = np.concatenate(cols, axis=1)
    return np.tile(a, (8, 1)).astype(np.int16)


def _assign_cores(nbund):
    """LPT assignment of nodes to cores balancing bundle counts."""
    order = np.argsort(-nbund, kind="stable")
    loads = np.zeros(CORES, np.int64)
    counts = np.zeros(CORES, np.int64)
    core_of = np.empty(N, np.int64)
    nrounds = (N + CORES - 1) // CORES
    for rnd in range(nrounds):
        chunk = order[rnd * CORES:(rnd + 1) * CORES]
        corder = np.argsort(loads, kind="stable")[:len(chunk)]
        core_of[chunk] = corder
        loads[corder] += nbund[chunk]
        counts[corder] += 1
    return core_of


def _pack_bins(nodes, nbund):
    """LPT deal of `nodes` (bundle counts nbund[nodes]) into R bins of
    <=CAP_NODES nodes and <=SLOTS_PER_RANGE bundles: rounds of R nodes
    (sorted desc) go to the currently least-loaded bins, which balances
    bundle load while keeping node counts equal. R is bumped until the
    bundle cap holds. Returns (bin_of_node, slot_of_node, nbins)."""
    nb = nbund[nodes]
    order = np.argsort(-nb, kind="stable")
    R = max(int(np.ceil(nb.sum() / SLOTS_PER_RANGE)),
            int(np.ceil(len(nodes) / CAP_NODES)))
    while True:
        loads = np.zeros(R, np.int64)
        counts = np.zeros(R, np.int64)
        bin_of = np.empty(len(nodes), np.int64)
        slot_of = np.empty(len(nodes), np.int64)
        nrounds = (len(nodes) + R - 1) // R
        for rnd in range(nrounds):
            chunk = order[rnd * R:(rnd + 1) * R]
            border = np.argsort(loads, kind="stable")[:len(chunk)]
            bin_of[chunk] = border
            slot_of[chunk] = counts[border]
            loads[border] += nb[chunk]
            counts[border] += 1
        if loads.max() <= SLOTS_PER_RANGE and counts.max() <= CAP_NODES:
            return bin_of, slot_of, R
        R += 1


def preprocess(x, edge_index):
    src = np.asarray(edge_index[0], dtype=np.int64)
    dst = np.asarray(edge_index[1], dtype=np.int64)
    deg = np.bincount(dst, minlength=N)
    recip = (1.0 / np.maximum(deg, 1)).astype(np.float32)
    nbund = (deg + LANES - 1) // LANES

    core_of = _assign_cores(nbund)

    pos_of_node = np.full(N, -1, np.int64)
    nbins_c = np.zeros(CORES, np.int64)
    for c in range(CORES):
        nodes = np.where(core_of == c)[0]
        bin_of, slot_of, nbins = _pack_bins(nodes, nbund)
        pos_of_node[nodes] = bin_of * 128 + slot_of
        nbins_c[c] = nbins
    RANGES = int(nbins_c.max())
    NPAD = RANGES * 128
    TOTBLK = RANGES * BPR
    groups = _make_groups(RANGES)

    xv = np.asarray(x, dtype=np.float32)
    cores = []
    for c in range(CORES):
        m = core_of[dst] == c
        s_e = src[m]
        pos_e = pos_of_node[dst[m]]
        o = np.argsort(pos_e, kind="stable")
        s_e = s_e[o]
        pos_e = pos_e[o]
        # rank of each edge within its dst run
        newd = np.r_[True, pos_e[1:] != pos_e[:-1]]
        starts = np.flatnonzero(newd)
        gid = np.cumsum(newd) - 1
        rank = np.arange(len(pos_e)) - starts[gid]
        lane = rank % LANES
        # bundle boundaries: new dst or new group-of-4
        newb = newd | ((rank % LANES == 0) & (rank > 0))
        bid = np.cumsum(newb) - 1
        B = int(newb.sum())
        if B + 1 > ROWS:
            raise OverflowError(f"table rows exhausted: {B + 1} > {ROWS}")
        b_pos = pos_e[np.flatnonzero(newb)]
        b_node = dst[m][o][np.flatnonzero(newb)]
        b_range = b_pos // 128
        b_slot = b_pos % 128
        bsrc = np.full((B, LANES), -1, np.int64)
        bsrc[bid, lane] = s_e
        cnt_r = np.bincount(b_range, minlength=RANGES)
        if cnt_r.max() > SLOTS_PER_RANGE:
            raise OverflowError(f"range overflow {cnt_r.max()}")
        base_r = np.concatenate([[0], np.cumsum(cnt_r)])
        slot_in_range = np.arange(B) - base_r[b_range]
        row_id = 1 + np.arange(B)

        idx_full = np.zeros(TOTBLK * 128, np.int16)
        tgt_full = np.full(TOTBLK * 128, 255, np.float32)
        gslot = b_range * SLOTS_PER_RANGE + slot_in_range
        idx_full[gslot] = row_id
        tgt_full[gslot] = b_slot

        call_streams = [idx_full[boff * 128:(boff + nblk) * 128]
                        for (_, _, boff, nblk) in groups]
        wrap = _wrap_idxs(call_streams)
        tgtT = np.ascontiguousarray(
            tgt_full.reshape(TOTBLK, 128).T).astype(NP_BF16)

        own = np.full(NPAD, -1, np.int64)
        nodes = np.where(core_of == c)[0]
        own[pos_of_node[nodes]] = nodes

        used = own >= 0
        t = np.zeros((NPAD, D), np.float32)
        t[used] = xv[own[used]]
        xT = np.ascontiguousarray(t.T).astype(NP_BF16)

        cores.append(dict(wrap=wrap, tgt=tgtT, bsrc=bsrc,
                          brecip=recip[b_node].astype(np.float32),
                          row_id=row_id, own=own, xT=xT))

    def table_from(feats_by_node):
        out = []
        for c in range(CORES):
            cc = cores[c]
            t = np.zeros((ROWS, LANES * D), NP_F8)
            bsrc = cc["bsrc"]
            rr = cc["brecip"]
            rid = cc["row_id"]
            for ln in range(LANES):
                mm = bsrc[:, ln] >= 0
                vals = feats_by_node[bsrc[mm, ln]] * rr[mm, None]
                t[rid[mm], ln * D:(ln + 1) * D] = vals.astype(NP_F8)
            out.append(t)
        return out

    return cores, table_from, RANGES, NPAD, xv


def kernel(x, edge_index, W1_l, b1, W1_r, W2_l, b2, W2_r, _timing=None):
    cores, table_from, RANGES, NPAD, xv = preprocess(x, edge_index)

    if RANGES not in _prog_cache:
        _prog_cache[RANGES] = (build_program(1, RANGES),
                               build_program(2, RANGES))
    nc1, nc2 = _prog_cache[RANGES]

    def wmat(w):
        return np.asarray(w, dtype=np.float32).astype(NP_BF16)

    def bcol(b):
        return np.asarray(b, dtype=np.float32).reshape(128, 1)

    iota = np.ascontiguousarray(
        np.broadcast_to(np.arange(128, dtype=np.float32), (128, 128))
    ).astype(NP_BF16)
    tables1 = table_from(xv)
    maps1 = []
    for c in range(CORES):
        cc = cores[c]
        maps1.append(dict(table=tables1[c], idxs=cc["wrap"], tgt=cc["tgt"],
                          iota=iota, xT=cc["xT"], Wl=wmat(W1_l),
                          Wr=wmat(W1_r), bvec=bcol(b1)))
    r1 = bass_utils.run_bass_kernel_spmd(nc1, maps1, core_ids=list(range(CORES)))

    h_node = np.zeros((N, D), np.float32)
    for c in range(CORES):
        own = cores[c]["own"]
        used = own >= 0
        h_node[own[used]] = r1.results[c]["tout"].T[used]
    tables2 = table_from(h_node)

    maps2 = []
    for c in range(CORES):
        cc = cores[c]
        hT_own = np.asarray(r1.results[c]["tout"], dtype=np.float32).astype(NP_BF16)
        maps2.append(dict(table=tables2[c], idxs=cc["wrap"], tgt=cc["tgt"],
                          iota=iota, xT=hT_own, Wl=wmat(W2_l),
                          Wr=wmat(W2_r), bvec=bcol(b2)))
    r2 = bass_utils.run_bass_kernel_spmd(nc2, maps2, core_ids=list(range(CORES)))
    if _timing is not None:
        _timing["nc1"] = nc1
        _timing["nc2"] = nc2

    out = np.empty((N, D), np.float32)
    for c in range(CORES):
        own = cores[c]["own"]
        used = own >= 0
        out[own[used]] = r2.results[c]["tout"].T[used]
    return out
